# revision 3
# baseline (speedup 1.0000x reference)
"""Trainium2 Bass kernel for CubicSplineAutoregressiveSubsetTransform2d.

Computes, per element (B,C,H,W), a monotone cubic Hermite spline (nsf
cubic_spline forward) parameterized by 34 per-element params
(16 widths, 16 heights, 2 derivs), applied to two inputs x_lower/x_upper.

Algorithmic trick: the spline is monotone increasing, so instead of
searchsorted + gather we use the telescoping identity

    z(x) = sum_k sg_k*(D0_k + u_k*(bc_k - aN_k*u_k)),
    sg_k = clamp(x - CW_{k-1}, 0, w_k),  u_k = sg_k / w_k

where full bins contribute exactly h_k and the partial bin contributes the
local cubic. No masks, no gathers.

Precision split (validated numerically): the knot-position path
(exp_w -> sum -> 1/sum -> widths -> cumsum -> x - cw) must be fp32 (position
errors are amplified by spline slopes up to ~3000x near narrow bins);
everything else is h-scaled and safe in fp16, which gives the DVE 2x_1p
tensor_tensor mode (~0.7 ns/elem vs ~1.4 fp32).

Memory layout (m-major): element e = p*M + m for partition p, so every
DRAM<->SBUF transfer is one contiguous run per partition. x_lower/x_upper
load once as [128, M]; z accumulates in a resident [128, 2, M] tile and
stores as two single DMAs (the v1 per-tile strided stores generated 4-byte
DMA packets that serialized all 16 SDMA engines for ~2.2ms). DMAs issue
from the Sync engine (HWDGE) to keep the Scalar engine free for ACT work.

Sharding: pure data-parallel over batch dim across 8 NeuronCores.
"""

import sys

import numpy as np

for _p in ("/opt/trn_rl_repo",):
    if _p not in sys.path:
        sys.path.insert(0, _p)

import concourse.bass as bass
import concourse.bacc as bacc
import concourse.mybir as mybir
from concourse import tile
from concourse.bass_utils import run_bass_kernel_spmd

F32 = mybir.dt.float32
F16 = mybir.dt.float16
AX = mybir.AxisListType
OP = mybir.AluOpType
ACT = mybir.ActivationFunctionType

B, C, H, W, K = 32, 3, 128, 128, 16
N_CORES = 8
MIN_BIN = 1e-3
SCALE = 1.0 - MIN_BIN * K  # 0.984


def build_program(n_elems: int, S: int = 96):
    """Build the SPMD Bass program for one core processing n_elems elements."""
    P = 128
    per_tile = P * S
    assert n_elems % per_tile == 0
    T = n_elems // per_tile
    M = T * S  # elements per partition

    nc = bacc.Bacc()
    xl_d = nc.dram_tensor("x_lower", [n_elems], F32, kind="ExternalInput")
    xu_d = nc.dram_tensor("x_upper", [n_elems], F32, kind="ExternalInput")
    pp_d = nc.dram_tensor("elementwise_params", [n_elems, 2 * K + 2], F32,
                          kind="ExternalInput")
    zl_d = nc.dram_tensor("z_lower", [n_elems], F32, kind="ExternalOutput")
    zu_d = nc.dram_tensor("z_upper", [n_elems], F32, kind="ExternalOutput")

    # m-major: element e = p*M + (t*S + s)
    pr = pp_d[:].rearrange("(p t s) k -> t p s k", p=P, t=T, s=S)
    xlr = xl_d[:].rearrange("(p m) -> p m", p=P)
    xur = xu_d[:].rearrange("(p m) -> p m", p=P)
    zlr = zl_d[:].rearrange("(p m) -> p m", p=P)
    zur = zu_d[:].rearrange("(p m) -> p m", p=P)

    with tile.TileContext(nc) as tc:
        with tc.tile_pool(name="cst", bufs=1) as cst, \
             tc.tile_pool(name="io", bufs=2) as io, \
             tc.tile_pool(name="wk", bufs=1) as wk, \
             tc.tile_pool(name="ac", bufs=2) as ac:
            # segment mask for the in-tile fp32 cumsum scan: 0 at k=0, 1 else
            segm = cst.tile([P, S, K], F32, tag="segm")
            nc.vector.memset(segm[:], 1.0)
            nc.vector.memset(segm[:, :, 0:1], 0.0)
            # resident inputs / outputs ([128, M]: one contiguous run per
            # partition in DRAM -> minimal DMA descriptor count)
            xlf = cst.tile([P, M], F32, tag="xlf")
            nc.sync.dma_start(out=xlf[:], in_=xlr)
            xuf = cst.tile([P, M], F32, tag="xuf")
            nc.sync.dma_start(out=xuf[:], in_=xur)
            zall = cst.tile([P, 2, M], F32, tag="zall")

            for t in range(T):
                sl = slice(t * S, (t + 1) * S)
                raw = io.tile([P, S, 34], F32, tag="raw")
                nc.sync.dma_start(out=raw[:], in_=pr[t])

                # =========== W path: fp32 =================================
                ew = ac.tile([P, S, K], F32, tag="ew")
                nc.scalar.activation(ew[:], raw[:, :, 0:K], ACT.Exp)
                Sw = wk.tile([P, S], F32, tag="Sw")
                nc.vector.reduce_sum(Sw[:], ew[:], axis=AX.X)
                rSw = wk.tile([P, S], F32, tag="rSw")
                nc.vector.reciprocal_approx_fast(rSw[:], Sw[:])
                nc.vector.tensor_scalar(rSw[:], rSw[:], SCALE, None, OP.mult)
                wt32 = wk.tile([P, S, K], F32, tag="wt32")
                rSw_b = rSw[:].unsqueeze(2).broadcast_to([P, S, K])
                nc.vector.tensor_tensor(wt32[:], ew[:], rSw_b, OP.mult)
                nc.scalar.activation(wt32[:], wt32[:], ACT.Copy, bias=MIN_BIN)
                cw = wk.tile([P, S, K], F32, tag="cw")
                nc.vector.tensor_tensor_scan(
                    cw[:].rearrange("p s k -> p (s k)"),
                    segm[:].rearrange("p s k -> p (s k)"),
                    wt32[:].rearrange("p s k -> p (s k)"),
                    0.0, OP.mult, OP.add)
                rw32 = wk.tile([P, S, K], F32, tag="rw32")
                nc.vector.reciprocal_approx_fast(rw32[:], wt32[:])
                # fp16 downcasts (ACT, contiguous writes)
                wt16 = wk.tile([P, S, K], F16, tag="wt16")
                nc.scalar.copy(wt16[:], wt32[:])
                rw16 = wk.tile([P, S, K], F16, tag="rw16")
                nc.scalar.copy(rw16[:], rw32[:])

                # =========== H path: fp16 =================================
                eh = wk.tile([P, S, K], F16, tag="eh")
                nc.scalar.activation(eh[:], raw[:, :, K:2 * K], ACT.Exp)
                hs = wk.tile([P, S, K // 2], F16, tag="hs")
                nc.vector.tensor_tensor(hs[:], eh[:, :, 0:8], eh[:, :, 8:16],
                                        OP.add)
                nc.vector.tensor_tensor(hs[:, :, 0:4], hs[:, :, 0:4],
                                        hs[:, :, 4:8], OP.add)
                nc.vector.tensor_tensor(hs[:, :, 0:2], hs[:, :, 0:2],
                                        hs[:, :, 2:4], OP.add)
                Sh = wk.tile([P, S], F32, tag="Sh")
                nc.vector.tensor_tensor(Sh[:], hs[:, :, 0], hs[:, :, 1],
                                        OP.add)
                rSh32 = wk.tile([P, S], F32, tag="rSh32")
                nc.vector.reciprocal_approx_fast(rSh32[:], Sh[:])
                rSh = wk.tile([P, S], F16, tag="rSh")
                nc.vector.tensor_scalar(rSh[:], rSh32[:], SCALE, None, OP.mult)
                ht = wk.tile([P, S, K], F16, tag="ht")
                rSh_b = rSh[:].unsqueeze(2).broadcast_to([P, S, K])
                nc.vector.tensor_tensor(ht[:], eh[:], rSh_b, OP.mult)
                nc.vector.tensor_scalar(ht[:], ht[:], MIN_BIN, None, OP.add)

                # =========== slopes + derivatives (fp16) ===================
                st_ = wk.tile([P, S, K], F16, tag="st")
                nc.vector.tensor_tensor(st_[:], ht[:], rw16[:], OP.mult)
                # aligned copies of the +1-shifted slices (ACT)
                stR = wk.tile([P, S, K], F16, tag="stR")
                nc.scalar.copy(stR[:, :, 0:K - 1], st_[:, :, 1:K])
                wtR = wk.tile([P, S, K], F16, tag="wtR")
                nc.scalar.copy(wtR[:, :, 0:K - 1], wt16[:, :, 1:K])
                sL = st_[:, :, 0:K - 1]
                wL = wt16[:, :, 0:K - 1]
                sR = stR[:, :, 0:K - 1]
                wR = wtR[:, :, 0:K - 1]
                m1 = wk.tile([P, S, K], F16, tag="m1")
                nc.vector.tensor_tensor(m1[:, :, 0:K - 1], sL, sR, OP.min)
                t1 = wk.tile([P, S, K], F16, tag="t1")
                nc.vector.tensor_tensor(t1[:, :, 0:K - 1], wR, sL, OP.mult)
                t2 = wk.tile([P, S, K], F16, tag="t2")
                nc.vector.tensor_tensor(t2[:, :, 0:K - 1], wL, sR, OP.mult)
                nc.vector.tensor_tensor(t1[:, :, 0:K - 1], t1[:, :, 0:K - 1],
                                        t2[:, :, 0:K - 1], OP.add)
                den32 = wk.tile([P, S, K], F32, tag="den32")
                nc.vector.tensor_tensor(den32[:, :, 0:K - 1], wL, wR, OP.add)
                rdn32 = wk.tile([P, S, K], F32, tag="rdn32")
                nc.vector.reciprocal_approx_fast(rdn32[:, :, 0:K - 1],
                                                 den32[:, :, 0:K - 1])
                rdn16 = wk.tile([P, S, K], F16, tag="rdn16")
                nc.scalar.copy(rdn16[:, :, 0:K - 1], rdn32[:, :, 0:K - 1])
                nc.vector.tensor_tensor(t1[:, :, 0:K - 1], t1[:, :, 0:K - 1],
                                        rdn16[:, :, 0:K - 1], OP.mult)
                m1d = wk.tile([P, S, K], F16, tag="m1d")
                nc.scalar.mul(m1d[:, :, 0:K - 1], m1[:, :, 0:K - 1], 2.0)
                # dlt padded to 18 so D0 slices stay 4B-aligned
                dlt = wk.tile([P, S, K + 2], F16, tag="dlt")
                nc.vector.tensor_tensor(dlt[:, :, 1:K], m1d[:, :, 0:K - 1],
                                        t1[:, :, 0:K - 1], OP.min)
                e01 = wk.tile([P, S, 2], F16, tag="e01")
                nc.scalar.activation(e01[:], raw[:, :, 2 * K:2 * K + 2],
                                     ACT.Tanh, scale=0.5)
                nc.vector.tensor_scalar(e01[:], e01[:], 1.5, 1.5,
                                        OP.mult, OP.add)
                nc.vector.tensor_tensor(dlt[:, :, 0:1], e01[:, :, 0:1],
                                        st_[:, :, 0:1], OP.mult)
                nc.vector.tensor_tensor(dlt[:, :, K:K + 1], e01[:, :, 1:2],
                                        st_[:, :, K - 1:K], OP.mult)

                # =========== Hermite coefficients ==========================
                D0 = dlt[:, :, 0:K]
                # aligned copy of D1 (ACT), then all coeff ops run 2x
                d1c = wk.tile([P, S, K], F16, tag="d1c")
                nc.scalar.copy(d1c[:], dlt[:, :, 1:K + 1])
                ds = wk.tile([P, S, K], F16, tag="ds")
                nc.vector.tensor_tensor(ds[:], D0, d1c[:], OP.add)
                st2 = wk.tile([P, S, K], F16, tag="st2")
                nc.scalar.mul(st2[:], st_[:], 2.0)
                aN = wk.tile([P, S, K], F16, tag="aN")
                nc.vector.tensor_tensor(aN[:], st2[:], ds[:], OP.subtract)
                sm = wk.tile([P, S, K], F16, tag="sm")
                nc.vector.tensor_tensor(sm[:], st_[:], D0, OP.subtract)
                bc = wk.tile([P, S, K], F16, tag="bc")
                nc.vector.tensor_tensor(bc[:], aN[:], sm[:], OP.add)

                # =========== evaluate both x in one [P,2,S,K] stream =======
                # pair dim OUTSIDE S so each z half is s-contiguous
                tt2 = wk.tile([P, 2, S, K], F16, tag="tt2")
                for j, xf in ((0, xlf), (1, xuf)):
                    x_b = xf[:, sl].unsqueeze(2).broadcast_to([P, S, K - 1])
                    nc.vector.tensor_tensor(tt2[:, j, :, 1:K], x_b,
                                            cw[:, :, 0:K - 1], OP.subtract)
                    nc.vector.tensor_copy(tt2[:, j, :, 0:1],
                                          xf[:, sl].unsqueeze(2))
                nc.scalar.activation(tt2[:], tt2[:], ACT.Relu)
                wt_b = wt16[:].unsqueeze(1).broadcast_to([P, 2, S, K])
                sg2 = wk.tile([P, 2, S, K], F16, tag="sg2")
                nc.vector.tensor_tensor(sg2[:], tt2[:], wt_b, OP.min)
                u2 = wk.tile([P, 2, S, K], F16, tag="u2")
                rw_b = rw16[:].unsqueeze(1).broadcast_to([P, 2, S, K])
                nc.vector.tensor_tensor(u2[:], sg2[:], rw_b, OP.mult)
                aN_b = aN[:].unsqueeze(1).broadcast_to([P, 2, S, K])
                bc_b = bc[:].unsqueeze(1).broadcast_to([P, 2, S, K])
                D0_b = D0.unsqueeze(1).broadcast_to([P, 2, S, K])
                hv = wk.tile([P, 2, S, K], F16, tag="hv")
                nc.vector.tensor_tensor(hv[:], aN_b, u2[:], OP.mult)
                nc.vector.tensor_tensor(hv[:], bc_b, hv[:], OP.subtract)
                nc.vector.tensor_tensor(hv[:], hv[:], u2[:], OP.mult)
                nc.vector.tensor_tensor(hv[:], hv[:], D0_b, OP.add)
                nc.vector.tensor_tensor(hv[:], hv[:], sg2[:], OP.mult)
                # tree to 4, then one reduce into the resident z tile
                nc.vector.tensor_tensor(hv[:, :, :, 0:8], hv[:, :, :, 0:8],
                                        hv[:, :, :, 8:16], OP.add)
                nc.vector.tensor_tensor(hv[:, :, :, 0:4], hv[:, :, :, 0:4],
                                        hv[:, :, :, 4:8], OP.add)
                zt = zall[:, :, sl]
                nc.vector.reduce_sum(zt, hv[:, :, :, 0:4], axis=AX.X)
                nc.vector.tensor_scalar(zt, zt, 1.0, 0.0, OP.min, OP.max)

            nc.sync.dma_start(out=zlr, in_=zall[:, 0, :])
            nc.sync.dma_start(out=zur, in_=zall[:, 1, :])
    nc.finalize()
    return nc


_PROGRAM_CACHE = {}


def _get_program(n_elems, S=96):
    key = (n_elems, S)
    if key not in _PROGRAM_CACHE:
        _PROGRAM_CACHE[key] = build_program(n_elems, S)
    return _PROGRAM_CACHE[key]


def kernel(x_lower, x_upper, elementwise_params):
    x_lower = np.ascontiguousarray(x_lower, dtype=np.float32)
    x_upper = np.ascontiguousarray(x_upper, dtype=np.float32)
    elementwise_params = np.ascontiguousarray(elementwise_params,
                                              dtype=np.float32)
    Bb = x_lower.shape[0]
    per = Bb // N_CORES
    n_elems = per * C * H * W

    nc = _get_program(n_elems)
    in_maps = []
    for c in range(N_CORES):
        sl = slice(c * per, (c + 1) * per)
        in_maps.append({
            "x_lower": x_lower[sl].reshape(n_elems),
            "x_upper": x_upper[sl].reshape(n_elems),
            "elementwise_params": elementwise_params[sl].reshape(n_elems, 34),
        })
    res = run_bass_kernel_spmd(nc, in_maps, list(range(N_CORES)))
    zl = np.concatenate([r["z_lower"].reshape(per, C, H, W)
                         for r in res.results], axis=0)
    zu = np.concatenate([r["z_upper"].reshape(per, C, H, W)
                         for r in res.results], axis=0)
    return zl, zu


if __name__ == "__main__":
    rng = np.random.default_rng(0)
    xl = rng.random((B, C, H, W), dtype=np.float32)
    xu = rng.random((B, C, H, W), dtype=np.float32)
    pp = rng.standard_normal((B, C, H, W, 34), dtype=np.float32)
    zl, zu = kernel(x_lower=xl, x_upper=xu, elementwise_params=pp)
    print("ok", zl.shape, zu.shape, zl.min(), zl.max())


# revision 5
# speedup vs baseline: 1.0317x; 1.0317x over previous
"""Trainium2 Bass kernel for CubicSplineAutoregressiveSubsetTransform2d.

Computes, per element (B,C,H,W), a monotone cubic Hermite spline (nsf
cubic_spline forward) parameterized by 34 per-element params
(16 widths, 16 heights, 2 derivs), applied to two inputs x_lower/x_upper.

Algorithmic trick: the spline is monotone increasing, so instead of
searchsorted + gather we use the telescoping identity

    z(x) = sum_k sg_k*(D0_k + u_k*(bc_k - aN_k*u_k)),
    sg_k = clamp(x - CW_{k-1}, 0, w_k),  u_k = sg_k / w_k

where full bins contribute exactly h_k and the partial bin contributes the
local cubic. No masks, no gathers.

Precision split (validated numerically): the knot-position path
(exp_w -> sum -> 1/sum -> widths -> cumsum -> x - cw) must be fp32 (position
errors are amplified by spline slopes up to ~3000x near narrow bins);
everything else is h-scaled and safe in fp16 (DVE 2x_1p tensor_tensor mode).

Engine split: two-source elementwise work lives on DVE (the only engine
that can run TENSOR_TENSOR); all single-source work (exp/tanh/relu,
up/downcasts, shifted-slice copies, +const biases) on the Scalar/ACT
engine; DMA issue on Sync (HWDGE). The two x evaluations share [P,2,S,K]
tiles so coefficients broadcast over the pair dim at the full 2x rate.

Memory layout (m-major): element e = p*M + m for partition p, so every
DRAM<->SBUF transfer is one contiguous run per partition; x loads once as
[128, M], z accumulates resident and stores as two single DMAs (the
original per-tile strided stores generated 4-byte DMA packets that
serialized all 16 SDMA engines for ~2.2ms).

Sharding: pure data-parallel over batch dim across 8 NeuronCores.
"""

import sys

import numpy as np

for _p in ("/opt/trn_rl_repo",):
    if _p not in sys.path:
        sys.path.insert(0, _p)

import concourse.bass as bass
import concourse.bacc as bacc
import concourse.mybir as mybir
from concourse import tile
from concourse.bass_utils import run_bass_kernel_spmd

F32 = mybir.dt.float32
F16 = mybir.dt.float16
AX = mybir.AxisListType
OP = mybir.AluOpType
ACT = mybir.ActivationFunctionType

B, C, H, W, K = 32, 3, 128, 128, 16
N_CORES = 8
MIN_BIN = 1e-3
SCALE = 1.0 - MIN_BIN * K  # 0.984


def build_program(n_elems: int, S: int = 96):
    """Build the SPMD Bass program for one core processing n_elems elements."""
    P = 128
    per_tile = P * S
    assert n_elems % per_tile == 0
    T = n_elems // per_tile
    M = T * S  # elements per partition

    nc = bacc.Bacc()
    xl_d = nc.dram_tensor("x_lower", [n_elems], F32, kind="ExternalInput")
    xu_d = nc.dram_tensor("x_upper", [n_elems], F32, kind="ExternalInput")
    pp_d = nc.dram_tensor("elementwise_params", [n_elems, 2 * K + 2], F32,
                          kind="ExternalInput")
    zl_d = nc.dram_tensor("z_lower", [n_elems], F32, kind="ExternalOutput")
    zu_d = nc.dram_tensor("z_upper", [n_elems], F32, kind="ExternalOutput")

    # m-major: element e = p*M + (t*S + s)
    pr = pp_d[:].rearrange("(p t s) k -> t p s k", p=P, t=T, s=S)
    xlr = xl_d[:].rearrange("(p m) -> p m", p=P)
    xur = xu_d[:].rearrange("(p m) -> p m", p=P)
    zlr = zl_d[:].rearrange("(p m) -> p m", p=P)
    zur = zu_d[:].rearrange("(p m) -> p m", p=P)

    with tile.TileContext(nc) as tc:
        with tc.tile_pool(name="cst", bufs=1) as cst, \
             tc.tile_pool(name="io", bufs=2) as io, \
             tc.tile_pool(name="wk", bufs=1) as wk, \
             tc.tile_pool(name="ac", bufs=2) as ac:
            # segment mask for the in-tile fp32 cumsum scan: 0 at k=0, 1 else
            segm = cst.tile([P, S, K], F32, tag="segm")
            nc.vector.memset(segm[:], 1.0)
            nc.vector.memset(segm[:, :, 0:1], 0.0)
            # resident inputs / outputs ([128, M]: one contiguous run per
            # partition in DRAM -> minimal DMA descriptor count)
            xlf = cst.tile([P, M], F32, tag="xlf")
            nc.sync.dma_start(out=xlf[:], in_=xlr)
            xuf = cst.tile([P, M], F32, tag="xuf")
            nc.sync.dma_start(out=xuf[:], in_=xur)
            zall = cst.tile([P, 2, M], F32, tag="zall")
            # cwx: exclusive-cumsum knots with a permanent 0 at k=0
            cwx = cst.tile([P, S, K], F32, tag="cwx")
            nc.vector.memset(cwx[:, :, 0:1], 0.0)

            for t in range(T):
                sl = slice(t * S, (t + 1) * S)
                raw = io.tile([P, S, 34], F32, tag="raw")
                nc.sync.dma_start(out=raw[:], in_=pr[t])

                # =========== W path: fp32 =================================
                ew = ac.tile([P, S, K], F32, tag="ew")
                nc.scalar.activation(ew[:], raw[:, :, 0:K], ACT.Exp)
                Sw = wk.tile([P, S], F32, tag="Sw")
                nc.vector.reduce_sum(Sw[:], ew[:], axis=AX.X)
                rSw = wk.tile([P, S], F32, tag="rSw")
                nc.vector.reciprocal_approx_fast(rSw[:], Sw[:])
                nc.vector.tensor_scalar(rSw[:], rSw[:], SCALE, None, OP.mult)
                wt32 = wk.tile([P, S, K], F32, tag="wt32")
                rSw_b = rSw[:].unsqueeze(2).broadcast_to([P, S, K])
                nc.vector.tensor_tensor(wt32[:], ew[:], rSw_b, OP.mult)
                nc.scalar.activation(wt32[:], wt32[:], ACT.Copy, bias=MIN_BIN)
                cw = wk.tile([P, S, K], F32, tag="cw")
                nc.vector.tensor_tensor_scan(
                    cw[:].rearrange("p s k -> p (s k)"),
                    segm[:].rearrange("p s k -> p (s k)"),
                    wt32[:].rearrange("p s k -> p (s k)"),
                    0.0, OP.mult, OP.add)
                # shifted knots (exclusive cumsum, 0 at k=0) for x - CW_{k-1}
                nc.scalar.copy(cwx[:, :, 1:K], cw[:, :, 0:K - 1])
                rw32 = wk.tile([P, S, K], F32, tag="rw32")
                nc.vector.reciprocal_approx_fast(rw32[:], wt32[:])
                # fp16 downcasts (ACT, contiguous writes)
                wt16 = wk.tile([P, S, K], F16, tag="wt16")
                nc.scalar.copy(wt16[:], wt32[:])
                rw16 = wk.tile([P, S, K], F16, tag="rw16")
                nc.scalar.copy(rw16[:], rw32[:])

                # =========== H path: fp16 =================================
                eh = wk.tile([P, S, K], F16, tag="eh")
                nc.scalar.activation(eh[:], raw[:, :, K:2 * K], ACT.Exp)
                hs = wk.tile([P, S, K // 2], F16, tag="hs")
                nc.vector.tensor_tensor(hs[:], eh[:, :, 0:8], eh[:, :, 8:16],
                                        OP.add)
                nc.vector.tensor_tensor(hs[:, :, 0:4], hs[:, :, 0:4],
                                        hs[:, :, 4:8], OP.add)
                Sh = wk.tile([P, S], F32, tag="Sh")
                nc.vector.reduce_sum(Sh[:], hs[:, :, 0:4], axis=AX.X)
                rSh32 = wk.tile([P, S], F32, tag="rSh32")
                nc.vector.reciprocal_approx_fast(rSh32[:], Sh[:])
                rSh = wk.tile([P, S], F16, tag="rSh")
                nc.vector.tensor_scalar(rSh[:], rSh32[:], SCALE, None, OP.mult)
                ht = wk.tile([P, S, K], F16, tag="ht")
                rSh_b = rSh[:].unsqueeze(2).broadcast_to([P, S, K])
                nc.vector.tensor_tensor(ht[:], eh[:], rSh_b, OP.mult)
                nc.scalar.activation(ht[:], ht[:], ACT.Copy, bias=MIN_BIN)

                # =========== slopes + derivatives (fp16) ===================
                st_ = wk.tile([P, S, K], F16, tag="st")
                nc.vector.tensor_tensor(st_[:], ht[:], rw16[:], OP.mult)
                # aligned copies of the +1-shifted slices (ACT)
                stR = wk.tile([P, S, K], F16, tag="stR")
                nc.scalar.copy(stR[:, :, 0:K - 1], st_[:, :, 1:K])
                wtR = wk.tile([P, S, K], F16, tag="wtR")
                nc.scalar.copy(wtR[:, :, 0:K - 1], wt16[:, :, 1:K])
                sL = st_[:, :, 0:K - 1]
                wL = wt16[:, :, 0:K - 1]
                sR = stR[:, :, 0:K - 1]
                wR = wtR[:, :, 0:K - 1]
                m1 = wk.tile([P, S, K], F16, tag="m1")
                nc.vector.tensor_tensor(m1[:, :, 0:K - 1], sL, sR, OP.min)
                t1 = wk.tile([P, S, K], F16, tag="t1")
                nc.vector.tensor_tensor(t1[:, :, 0:K - 1], wR, sL, OP.mult)
                t2 = wk.tile([P, S, K], F16, tag="t2")
                nc.vector.tensor_tensor(t2[:, :, 0:K - 1], wL, sR, OP.mult)
                nc.vector.tensor_tensor(t1[:, :, 0:K - 1], t1[:, :, 0:K - 1],
                                        t2[:, :, 0:K - 1], OP.add)
                den16 = wk.tile([P, S, K], F16, tag="den16")
                nc.vector.tensor_tensor(den16[:, :, 0:K - 1], wL, wR, OP.add)
                den32 = wk.tile([P, S, K], F32, tag="den32")
                nc.scalar.copy(den32[:, :, 0:K - 1], den16[:, :, 0:K - 1])
                rdn32 = wk.tile([P, S, K], F32, tag="rdn32")
                nc.vector.reciprocal_approx_fast(rdn32[:, :, 0:K - 1],
                                                 den32[:, :, 0:K - 1])
                rdn16 = wk.tile([P, S, K], F16, tag="rdn16")
                nc.scalar.copy(rdn16[:, :, 0:K - 1], rdn32[:, :, 0:K - 1])
                nc.vector.tensor_tensor(t1[:, :, 0:K - 1], t1[:, :, 0:K - 1],
                                        rdn16[:, :, 0:K - 1], OP.mult)
                m1d = wk.tile([P, S, K], F16, tag="m1d")
                nc.scalar.mul(m1d[:, :, 0:K - 1], m1[:, :, 0:K - 1], 2.0)
                # dlt padded to 18 so D0 slices stay 4B-aligned
                dlt = wk.tile([P, S, K + 2], F16, tag="dlt")
                nc.vector.tensor_tensor(dlt[:, :, 1:K], m1d[:, :, 0:K - 1],
                                        t1[:, :, 0:K - 1], OP.min)
                e01 = wk.tile([P, S, 2], F16, tag="e01")
                nc.scalar.activation(e01[:], raw[:, :, 2 * K:2 * K + 2],
                                     ACT.Tanh, scale=0.5)
                nc.vector.tensor_scalar(e01[:], e01[:], 1.5, 1.5,
                                        OP.mult, OP.add)
                nc.vector.tensor_tensor(dlt[:, :, 0:1], e01[:, :, 0:1],
                                        st_[:, :, 0:1], OP.mult)
                nc.vector.tensor_tensor(dlt[:, :, K:K + 1], e01[:, :, 1:2],
                                        st_[:, :, K - 1:K], OP.mult)

                # =========== Hermite coefficients ==========================
                D0 = dlt[:, :, 0:K]
                # aligned copy of D1 (ACT), then all coeff ops run 2x
                d1c = wk.tile([P, S, K], F16, tag="d1c")
                nc.scalar.copy(d1c[:], dlt[:, :, 1:K + 1])
                # aN = 2st - D0 - D1 = (st-D0) + (st-D1); bc = aN + (st-D0)
                sm = wk.tile([P, S, K], F16, tag="sm")
                nc.vector.tensor_tensor(sm[:], st_[:], D0, OP.subtract)
                sm1 = wk.tile([P, S, K], F16, tag="sm1")
                nc.vector.tensor_tensor(sm1[:], st_[:], d1c[:], OP.subtract)
                aN = wk.tile([P, S, K], F16, tag="aN")
                nc.vector.tensor_tensor(aN[:], sm[:], sm1[:], OP.add)
                bc = wk.tile([P, S, K], F16, tag="bc")
                nc.vector.tensor_tensor(bc[:], aN[:], sm[:], OP.add)

                # =========== evaluate both x in one [P,2,S,K] stream =======
                tt2 = wk.tile([P, 2, S, K], F16, tag="tt2")
                for j, xf in ((0, xlf), (1, xuf)):
                    x_b = xf[:, sl].unsqueeze(2).broadcast_to([P, S, K])
                    nc.vector.tensor_tensor(tt2[:, j], x_b, cwx[:],
                                            OP.subtract)
                nc.scalar.activation(tt2[:], tt2[:], ACT.Relu)
                wt_b = wt16[:].unsqueeze(1).broadcast_to([P, 2, S, K])
                sg2 = wk.tile([P, 2, S, K], F16, tag="sg2")
                nc.vector.tensor_tensor(sg2[:], tt2[:], wt_b, OP.min)
                u2 = wk.tile([P, 2, S, K], F16, tag="u2")
                rw_b = rw16[:].unsqueeze(1).broadcast_to([P, 2, S, K])
                nc.vector.tensor_tensor(u2[:], sg2[:], rw_b, OP.mult)
                aN_b = aN[:].unsqueeze(1).broadcast_to([P, 2, S, K])
                bc_b = bc[:].unsqueeze(1).broadcast_to([P, 2, S, K])
                D0_b = D0.unsqueeze(1).broadcast_to([P, 2, S, K])
                hv = wk.tile([P, 2, S, K], F16, tag="hv")
                nc.vector.tensor_tensor(hv[:], aN_b, u2[:], OP.mult)
                nc.vector.tensor_tensor(hv[:], bc_b, hv[:], OP.subtract)
                nc.vector.tensor_tensor(hv[:], hv[:], u2[:], OP.mult)
                nc.vector.tensor_tensor(hv[:], hv[:], D0_b, OP.add)
                nc.vector.tensor_tensor(hv[:], hv[:], sg2[:], OP.mult)
                # tree to 4, then one reduce into the resident z tile
                nc.vector.tensor_tensor(hv[:, :, :, 0:8], hv[:, :, :, 0:8],
                                        hv[:, :, :, 8:16], OP.add)
                nc.vector.tensor_tensor(hv[:, :, :, 0:4], hv[:, :, :, 0:4],
                                        hv[:, :, :, 4:8], OP.add)
                zt = zall[:, :, sl]
                nc.vector.reduce_sum(zt, hv[:, :, :, 0:4], axis=AX.X)
                nc.vector.tensor_scalar(zt, zt, 1.0, 0.0, OP.min, OP.max)

            nc.sync.dma_start(out=zlr, in_=zall[:, 0, :])
            nc.sync.dma_start(out=zur, in_=zall[:, 1, :])
    nc.finalize()
    return nc


_PROGRAM_CACHE = {}


def _get_program(n_elems, S=96):
    key = (n_elems, S)
    if key not in _PROGRAM_CACHE:
        _PROGRAM_CACHE[key] = build_program(n_elems, S)
    return _PROGRAM_CACHE[key]


def kernel(x_lower, x_upper, elementwise_params):
    x_lower = np.ascontiguousarray(x_lower, dtype=np.float32)
    x_upper = np.ascontiguousarray(x_upper, dtype=np.float32)
    elementwise_params = np.ascontiguousarray(elementwise_params,
                                              dtype=np.float32)
    Bb = x_lower.shape[0]
    per = Bb // N_CORES
    n_elems = per * C * H * W

    nc = _get_program(n_elems)
    in_maps = []
    for c in range(N_CORES):
        sl = slice(c * per, (c + 1) * per)
        in_maps.append({
            "x_lower": x_lower[sl].reshape(n_elems),
            "x_upper": x_upper[sl].reshape(n_elems),
            "elementwise_params": elementwise_params[sl].reshape(n_elems, 34),
        })
    res = run_bass_kernel_spmd(nc, in_maps, list(range(N_CORES)))
    zl = np.concatenate([r["z_lower"].reshape(per, C, H, W)
                         for r in res.results], axis=0)
    zu = np.concatenate([r["z_upper"].reshape(per, C, H, W)
                         for r in res.results], axis=0)
    return zl, zu


if __name__ == "__main__":
    rng = np.random.default_rng(0)
    xl = rng.random((B, C, H, W), dtype=np.float32)
    xu = rng.random((B, C, H, W), dtype=np.float32)
    pp = rng.standard_normal((B, C, H, W, 34), dtype=np.float32)
    zl, zu = kernel(x_lower=xl, x_upper=xu, elementwise_params=pp)
    print("ok", zl.shape, zu.shape, zl.min(), zl.max())


# revision 10
# speedup vs baseline: 1.0477x; 1.0155x over previous
"""Trainium2 Bass kernel for CubicSplineAutoregressiveSubsetTransform2d.

Computes, per element (B,C,H,W), a monotone cubic Hermite spline (nsf
cubic_spline forward) parameterized by 34 per-element params
(16 widths, 16 heights, 2 derivs), applied to two inputs x_lower/x_upper.

Algorithmic trick: the spline is monotone increasing, so instead of
searchsorted + gather we use the telescoping identity

    z(x) = sum_k sg_k*(D0_k + u_k*(bc_k - aN_k*u_k)),
    sg_k = clamp(x - CW_{k-1}, 0, w_k),  u_k = sg_k / w_k

where full bins contribute exactly h_k and the partial bin contributes the
local cubic. No masks, no gathers.

Precision split (validated numerically): the knot-position path
(exp_w -> sum -> 1/sum -> widths -> cumsum -> x - cw) must be fp32 (position
errors are amplified by spline slopes up to ~3000x near narrow bins);
everything else is h-scaled and safe in fp16 (DVE 2x_1p tensor_tensor mode).

Engine split: two-source elementwise work lives on DVE (the only engine
that can run TENSOR_TENSOR); all single-source work (exp/tanh/relu,
up/downcasts, shifted-slice copies, +const biases) on the Scalar/ACT
engine; DMA issue on Sync (HWDGE). The two x evaluations share [P,2,S,K]
tiles so coefficients broadcast over the pair dim at the full 2x rate.

Memory layout (m-major): element e = p*M + m for partition p, so every
DRAM<->SBUF transfer is one contiguous run per partition; x loads once as
[128, M], z accumulates resident and stores as two single DMAs (the
original per-tile strided stores generated 4-byte DMA packets that
serialized all 16 SDMA engines for ~2.2ms).

Sharding: pure data-parallel over batch dim across 8 NeuronCores.
"""

import sys

import numpy as np

for _p in ("/opt/trn_rl_repo",):
    if _p not in sys.path:
        sys.path.insert(0, _p)

import concourse.bass as bass
import concourse.bacc as bacc
import concourse.mybir as mybir
from concourse import tile
from concourse.bass_utils import run_bass_kernel_spmd

F32 = mybir.dt.float32
F16 = mybir.dt.float16
AX = mybir.AxisListType
OP = mybir.AluOpType
ACT = mybir.ActivationFunctionType

B, C, H, W, K = 32, 3, 128, 128, 16
N_CORES = 8
MIN_BIN = 1e-3
SCALE = 1.0 - MIN_BIN * K  # 0.984


def build_program(n_elems: int, S: int = 96):
    """Build the SPMD Bass program for one core processing n_elems elements."""
    P = 128
    per_tile = P * S
    assert n_elems % per_tile == 0
    T = n_elems // per_tile
    M = T * S  # elements per partition

    nc = bacc.Bacc()
    xl_d = nc.dram_tensor("x_lower", [n_elems], F32, kind="ExternalInput")
    xu_d = nc.dram_tensor("x_upper", [n_elems], F32, kind="ExternalInput")
    pp_d = nc.dram_tensor("elementwise_params", [n_elems, 2 * K + 2], F32,
                          kind="ExternalInput")
    zl_d = nc.dram_tensor("z_lower", [n_elems], F32, kind="ExternalOutput")
    zu_d = nc.dram_tensor("z_upper", [n_elems], F32, kind="ExternalOutput")

    # m-major: element e = p*M + (t*S + s)
    pr = pp_d[:].rearrange("(p t s) k -> t p s k", p=P, t=T, s=S)
    xlr = xl_d[:].rearrange("(p m) -> p m", p=P)
    xur = xu_d[:].rearrange("(p m) -> p m", p=P)
    zlr = zl_d[:].rearrange("(p m) -> p m", p=P)
    zur = zu_d[:].rearrange("(p m) -> p m", p=P)

    with tile.TileContext(nc) as tc:
        with tc.tile_pool(name="cst", bufs=1) as cst, \
             tc.tile_pool(name="io", bufs=2) as io, \
             tc.tile_pool(name="wk", bufs=1) as wk, \
             tc.tile_pool(name="ac", bufs=2) as ac:
            # segment mask for the in-tile fp32 cumsum scan: 0 at k=0, 1 else
            segm = cst.tile([P, S, K], F32, tag="segm")
            nc.vector.memset(segm[:], 1.0)
            nc.vector.memset(segm[:, :, 0:1], 0.0)
            # resident inputs / outputs ([128, M]: one contiguous run per
            # partition in DRAM -> minimal DMA descriptor count)
            xlf = cst.tile([P, M], F32, tag="xlf")
            nc.sync.dma_start(out=xlf[:], in_=xlr)
            xuf = cst.tile([P, M], F32, tag="xuf")
            nc.sync.dma_start(out=xuf[:], in_=xur)
            zall = cst.tile([P, 2, M], F32, tag="zall")
            # cwx: exclusive-cumsum knots with a permanent 0 at k=0
            cwx = cst.tile([P, S, K], F32, tag="cwx")
            nc.vector.memset(cwx[:, :, 0:1], 0.0)

            for t in range(T):
                sl = slice(t * S, (t + 1) * S)
                raw = io.tile([P, S, 34], F32, tag="raw")
                nc.sync.dma_start(out=raw[:], in_=pr[t])

                # =========== W path: fp32 =================================
                ew = ac.tile([P, S, K], F32, tag="ew")
                nc.scalar.activation(ew[:], raw[:, :, 0:K], ACT.Exp)
                Sw = wk.tile([P, S], F32, tag="Sw")
                nc.vector.reduce_sum(Sw[:], ew[:], axis=AX.X)
                rSw = wk.tile([P, S], F32, tag="rSw")
                nc.vector.reciprocal_approx_fast(rSw[:], Sw[:])
                nc.vector.tensor_scalar(rSw[:], rSw[:], SCALE, None, OP.mult)
                wt32 = wk.tile([P, S, K], F32, tag="wt32")
                rSw_b = rSw[:].unsqueeze(2).broadcast_to([P, S, K])
                nc.vector.tensor_tensor(wt32[:], ew[:], rSw_b, OP.mult)
                nc.scalar.activation(wt32[:], wt32[:], ACT.Copy, bias=MIN_BIN)
                cw = wk.tile([P, S, K], F32, tag="cw")
                nc.vector.tensor_tensor_scan(
                    cw[:].rearrange("p s k -> p (s k)"),
                    segm[:].rearrange("p s k -> p (s k)"),
                    wt32[:].rearrange("p s k -> p (s k)"),
                    0.0, OP.mult, OP.add)
                # shifted knots (exclusive cumsum, 0 at k=0) for x - CW_{k-1}
                nc.scalar.copy(cwx[:, :, 1:K], cw[:, :, 0:K - 1])
                rw32 = wk.tile([P, S, K], F32, tag="rw32")
                nc.vector.reciprocal_approx_fast(rw32[:], wt32[:])
                # fp16 downcasts (ACT, contiguous writes)
                wt16 = wk.tile([P, S, K], F16, tag="wt16")
                nc.scalar.copy(wt16[:], wt32[:])
                rw16 = wk.tile([P, S, K], F16, tag="rw16")
                nc.scalar.copy(rw16[:], rw32[:])

                # =========== H path: fp16 =================================
                eh = wk.tile([P, S, K], F16, tag="eh")
                nc.scalar.activation(eh[:], raw[:, :, K:2 * K], ACT.Exp)
                hs = wk.tile([P, S, K // 2], F16, tag="hs")
                nc.vector.tensor_tensor(hs[:], eh[:, :, 0:8], eh[:, :, 8:16],
                                        OP.add)
                nc.vector.tensor_tensor(hs[:, :, 0:4], hs[:, :, 0:4],
                                        hs[:, :, 4:8], OP.add)
                Sh = wk.tile([P, S], F32, tag="Sh")
                nc.vector.reduce_sum(Sh[:], hs[:, :, 0:4], axis=AX.X)
                rSh32 = wk.tile([P, S], F32, tag="rSh32")
                nc.vector.reciprocal_approx_fast(rSh32[:], Sh[:])
                rSh = wk.tile([P, S], F16, tag="rSh")
                nc.vector.tensor_scalar(rSh[:], rSh32[:], SCALE, None, OP.mult)
                ht = wk.tile([P, S, K], F16, tag="ht")
                rSh_b = rSh[:].unsqueeze(2).broadcast_to([P, S, K])
                nc.vector.tensor_tensor(ht[:], eh[:], rSh_b, OP.mult)
                nc.scalar.activation(ht[:], ht[:], ACT.Copy, bias=MIN_BIN)

                # =========== slopes + derivatives (fp16) ===================
                st_ = wk.tile([P, S, K], F16, tag="st")
                nc.vector.tensor_tensor(st_[:], ht[:], rw16[:], OP.mult)
                # aligned copies of the +1-shifted slices (ACT)
                stR = wk.tile([P, S, K], F16, tag="stR")
                nc.scalar.copy(stR[:, :, 0:K - 1], st_[:, :, 1:K])
                wtR = wk.tile([P, S, K], F16, tag="wtR")
                nc.scalar.copy(wtR[:, :, 0:K - 1], wt16[:, :, 1:K])
                sL = st_[:, :, 0:K - 1]
                wL = wt16[:, :, 0:K - 1]
                sR = stR[:, :, 0:K - 1]
                wR = wtR[:, :, 0:K - 1]
                m1 = wk.tile([P, S, K], F16, tag="m1")
                nc.vector.tensor_tensor(m1[:, :, 0:K - 1], sL, sR, OP.min)
                t1 = wk.tile([P, S, K], F16, tag="t1")
                nc.vector.tensor_tensor(t1[:, :, 0:K - 1], wR, sL, OP.mult)
                t2 = wk.tile([P, S, K], F16, tag="t2")
                nc.vector.tensor_tensor(t2[:, :, 0:K - 1], wL, sR, OP.mult)
                nc.vector.tensor_tensor(t1[:, :, 0:K - 1], t1[:, :, 0:K - 1],
                                        t2[:, :, 0:K - 1], OP.add)
                den16 = wk.tile([P, S, K], F16, tag="den16")
                nc.vector.tensor_tensor(den16[:, :, 0:K - 1], wL, wR, OP.add)
                # buffer reuse: cw is dead after the cwx copy, wt32 after
                # rw32/wt16, t2 after the t1+t2 fold
                den32 = wk.tile([P, S, K], F32, tag="cw")
                nc.scalar.copy(den32[:, :, 0:K - 1], den16[:, :, 0:K - 1])
                rdn32 = wk.tile([P, S, K], F32, tag="wt32")
                nc.vector.reciprocal_approx_fast(rdn32[:, :, 0:K - 1],
                                                 den32[:, :, 0:K - 1])
                rdn16 = wk.tile([P, S, K], F16, tag="t2")
                nc.scalar.copy(rdn16[:, :, 0:K - 1], rdn32[:, :, 0:K - 1])
                nc.vector.tensor_tensor(t1[:, :, 0:K - 1], t1[:, :, 0:K - 1],
                                        rdn16[:, :, 0:K - 1], OP.mult)
                m1d = wk.tile([P, S, K], F16, tag="den16")
                nc.scalar.mul(m1d[:, :, 0:K - 1], m1[:, :, 0:K - 1], 2.0)
                # dlt padded to 18 so D0 slices stay 4B-aligned
                dlt = wk.tile([P, S, K + 2], F16, tag="dlt")
                nc.vector.tensor_tensor(dlt[:, :, 1:K], m1d[:, :, 0:K - 1],
                                        t1[:, :, 0:K - 1], OP.min)
                e01 = wk.tile([P, S, 2], F16, tag="e01")
                nc.scalar.activation(e01[:], raw[:, :, 2 * K:2 * K + 2],
                                     ACT.Tanh, scale=0.5)
                nc.vector.tensor_scalar(e01[:], e01[:], 1.5, 1.5,
                                        OP.mult, OP.add)
                nc.vector.tensor_tensor(dlt[:, :, 0:1], e01[:, :, 0:1],
                                        st_[:, :, 0:1], OP.mult)
                nc.vector.tensor_tensor(dlt[:, :, K:K + 1], e01[:, :, 1:2],
                                        st_[:, :, K - 1:K], OP.mult)

                # =========== Hermite coefficients ==========================
                D0 = dlt[:, :, 0:K]
                # aligned copy of D1 (ACT), then all coeff ops run 2x
                d1c = wk.tile([P, S, K], F16, tag="eh")
                nc.scalar.copy(d1c[:], dlt[:, :, 1:K + 1])
                # aN = 2st - D0 - D1 = (st-D0) + (st-D1); bc = aN + (st-D0)
                sm = wk.tile([P, S, K], F16, tag="sm")
                nc.vector.tensor_tensor(sm[:], st_[:], D0, OP.subtract)
                sm1 = wk.tile([P, S, K], F16, tag="stR")
                nc.vector.tensor_tensor(sm1[:], st_[:], d1c[:], OP.subtract)
                aN = wk.tile([P, S, K], F16, tag="aN")
                nc.vector.tensor_tensor(aN[:], sm[:], sm1[:], OP.add)
                bc = wk.tile([P, S, K], F16, tag="bc")
                nc.vector.tensor_tensor(bc[:], aN[:], sm[:], OP.add)

                # =========== evaluate both x in one [P,2,S,K] stream =======
                tt2 = wk.tile([P, 2, S, K], F16, tag="tt2")
                for j, xf in ((0, xlf), (1, xuf)):
                    x_b = xf[:, sl].unsqueeze(2).broadcast_to([P, S, K])
                    nc.vector.tensor_tensor(tt2[:, j], x_b, cwx[:],
                                            OP.subtract)
                nc.scalar.activation(tt2[:], tt2[:], ACT.Relu)
                wt_b = wt16[:].unsqueeze(1).broadcast_to([P, 2, S, K])
                sg2 = wk.tile([P, 2, S, K], F16, tag="sg2")
                nc.vector.tensor_tensor(sg2[:], tt2[:], wt_b, OP.min)
                u2 = wk.tile([P, 2, S, K], F16, tag="tt2")
                rw_b = rw16[:].unsqueeze(1).broadcast_to([P, 2, S, K])
                nc.vector.tensor_tensor(u2[:], sg2[:], rw_b, OP.mult)
                aN_b = aN[:].unsqueeze(1).broadcast_to([P, 2, S, K])
                bc_b = bc[:].unsqueeze(1).broadcast_to([P, 2, S, K])
                D0_b = D0.unsqueeze(1).broadcast_to([P, 2, S, K])
                hv = wk.tile([P, 2, S, K], F16, tag="hv")
                nc.vector.tensor_tensor(hv[:], aN_b, u2[:], OP.mult)
                nc.vector.tensor_tensor(hv[:], bc_b, hv[:], OP.subtract)
                nc.vector.tensor_tensor(hv[:], hv[:], u2[:], OP.mult)
                nc.vector.tensor_tensor(hv[:], hv[:], D0_b, OP.add)
                nc.vector.tensor_tensor(hv[:], hv[:], sg2[:], OP.mult)
                # tree to 4, then one reduce into the resident z tile
                nc.vector.tensor_tensor(hv[:, :, :, 0:8], hv[:, :, :, 0:8],
                                        hv[:, :, :, 8:16], OP.add)
                nc.vector.tensor_tensor(hv[:, :, :, 0:4], hv[:, :, :, 0:4],
                                        hv[:, :, :, 4:8], OP.add)
                zt = zall[:, :, sl]
                nc.vector.reduce_sum(zt, hv[:, :, :, 0:4], axis=AX.X)
                nc.vector.tensor_scalar(zt, zt, 1.0, 0.0, OP.min, OP.max)

            nc.sync.dma_start(out=zlr, in_=zall[:, 0, :])
            nc.sync.dma_start(out=zur, in_=zall[:, 1, :])
    nc.finalize()
    return nc


_PROGRAM_CACHE = {}


def _get_program(n_elems, S=128):
    key = (n_elems, S)
    if key not in _PROGRAM_CACHE:
        _PROGRAM_CACHE[key] = build_program(n_elems, S)
    return _PROGRAM_CACHE[key]


def kernel(x_lower, x_upper, elementwise_params):
    x_lower = np.ascontiguousarray(x_lower, dtype=np.float32)
    x_upper = np.ascontiguousarray(x_upper, dtype=np.float32)
    elementwise_params = np.ascontiguousarray(elementwise_params,
                                              dtype=np.float32)
    Bb = x_lower.shape[0]
    per = Bb // N_CORES
    n_elems = per * C * H * W

    nc = _get_program(n_elems)
    in_maps = []
    for c in range(N_CORES):
        sl = slice(c * per, (c + 1) * per)
        in_maps.append({
            "x_lower": x_lower[sl].reshape(n_elems),
            "x_upper": x_upper[sl].reshape(n_elems),
            "elementwise_params": elementwise_params[sl].reshape(n_elems, 34),
        })
    res = run_bass_kernel_spmd(nc, in_maps, list(range(N_CORES)))
    zl = np.concatenate([r["z_lower"].reshape(per, C, H, W)
                         for r in res.results], axis=0)
    zu = np.concatenate([r["z_upper"].reshape(per, C, H, W)
                         for r in res.results], axis=0)
    return zl, zu


if __name__ == "__main__":
    rng = np.random.default_rng(0)
    xl = rng.random((B, C, H, W), dtype=np.float32)
    xu = rng.random((B, C, H, W), dtype=np.float32)
    pp = rng.standard_normal((B, C, H, W, 34), dtype=np.float32)
    zl, zu = kernel(x_lower=xl, x_upper=xu, elementwise_params=pp)
    print("ok", zl.shape, zu.shape, zl.min(), zl.max())


# revision 20
# speedup vs baseline: 1.1298x; 1.0784x over previous
"""Trainium2 Bass kernel for CubicSplineAutoregressiveSubsetTransform2d.

Computes, per element (B,C,H,W), a monotone cubic Hermite spline (nsf
cubic_spline forward) parameterized by 34 per-element params
(16 widths, 16 heights, 2 derivs), applied to two inputs x_lower/x_upper.

Algorithmic trick: the spline is monotone increasing, so instead of
searchsorted + gather we use the telescoping identity

    z(x) = sum_k sg_k*(D0_k + u_k*(bc_k - aN_k*u_k)),
    sg_k = clamp(x - CW_{k-1}, 0, w_k),  u_k = sg_k / w_k

where full bins contribute exactly h_k and the partial bin contributes the
local cubic. No masks, no gathers.

Precision split (validated numerically): the knot-position path
(exp_w -> sum -> 1/sum -> widths -> cumsum -> x - cw) must be fp32 (position
errors are amplified by spline slopes up to ~3000x near narrow bins);
everything else is h-scaled and safe in fp16 (DVE 2x_1p tensor_tensor mode).

Engine split: two-source elementwise work lives on DVE (the only engine
that can run TENSOR_TENSOR); all single-source work (exp/tanh/relu,
up/downcasts, shifted-slice copies, +const biases) on the Scalar/ACT
engine; DMA issue on Sync (HWDGE). The two x evaluations share [P,2,S,K]
tiles so coefficients broadcast over the pair dim at the full 2x rate.

Memory layout (m-major): element e = p*M + m for partition p, so every
DRAM<->SBUF transfer is one contiguous run per partition; x loads once as
[128, M], z accumulates resident and stores as two single DMAs (the
original per-tile strided stores generated 4-byte DMA packets that
serialized all 16 SDMA engines for ~2.2ms).

Sharding: pure data-parallel over batch dim across 8 NeuronCores.
"""

import sys

import numpy as np

for _p in ("/opt/trn_rl_repo",):
    if _p not in sys.path:
        sys.path.insert(0, _p)

import concourse.bass as bass
import concourse.bacc as bacc
import concourse.mybir as mybir
from concourse import tile
from concourse import dve_ops as DO
from concourse.bass_utils import run_bass_kernel_spmd
from concourse.dve_spec import (AluOp, Bin, Idx, Spec, Src0, Src1, SubIdx,
                                C0, Zero, lower as spec_lower, maxx, scan)
from concourse.dve_table_gen import dve_ver_for
from concourse.dve_uop import DveOpSpec

F32 = mybir.dt.float32
F16 = mybir.dt.float16
AX = mybir.AxisListType
OP = mybir.AluOpType
ACT = mybir.ActivationFunctionType


def _register_dve_op(name, spec, subdim):
    """Register a custom DVE op at runtime (the repo's OPS table is a plain
    module-level list; the sha pin is computed here, same as compile would)."""
    for op in DO.OPS:
        if op.name == name:
            return op
    row = max(DO._SUB_OPCODE_FOR_NAME.values()) + 1
    assert row < 0x20
    DO._SUB_OPCODE_FOR_NAME[name] = row
    shas = {}
    for ver in ("v3", "v4"):
        s = DveOpSpec(name=name, opcode=row, uops=spec_lower(spec, ver=ver),
                      rd1_en=DO.has_src1(spec))
        shas[ver] = s.sha(ver)
    op = DO.DveOp(name, spec, subdim, shas)
    DO.OPS.append(op)
    DO.CUSTOM_DVE_SPECS[name] = spec
    return op


def _ref_scan_relu(in0, in1, c0, c1, c2):
    # in0: [P, S, N] centered shifted widths; in1: x broadcast; c0 = 1/N
    P, S, N = in0.shape
    ex = np.cumsum(in0.astype(np.float32).reshape(P, S * N), axis=1)
    ex = ex.reshape(P, S, N)
    k = np.arange(N, dtype=np.float32)[None, None, :]
    s = np.arange(S, dtype=np.float32)[None, :, None]
    i = s * N + k
    return np.maximum(in1.astype(np.float32) - ex + (s - i * c0), 0.0)


# ttr = relu(x - CW_{k-1}): CW from a stream-wide fp32 scan of centered
# widths (w - 1/16; segments sum to exactly 0 so the accumulator stays O(1)),
# de-centered by the exact (SubIdx - Idx/16) = -k/16 correction.
_scan_val = scan(AluOp.ADD, Src0)
_SCAN_RELU = Spec(
    body=maxx(
        Bin(AluOp.ADD,
            Bin(AluOp.SUBTRACT, Src1, _scan_val),
            Bin(AluOp.SUBTRACT, SubIdx, Bin(AluOp.MULTIPLY, Idx, C0))),
        Zero),
    reference=_ref_scan_relu,
)

B, C, H, W, K = 32, 3, 128, 128, 16
N_CORES = 8
MIN_BIN = 1e-3
SCALE = 1.0 - MIN_BIN * K  # 0.984


def build_program(n_elems: int, S: int = 96):
    """Build the SPMD Bass program for one core processing n_elems elements."""
    P = 128
    per_tile = P * S
    assert n_elems % per_tile == 0
    T = n_elems // per_tile
    M = T * S  # elements per partition

    nc = bacc.Bacc()
    xl_d = nc.dram_tensor("x_lower", [n_elems], F32, kind="ExternalInput")
    xu_d = nc.dram_tensor("x_upper", [n_elems], F32, kind="ExternalInput")
    pp_d = nc.dram_tensor("elementwise_params", [n_elems, 2 * K + 2], F32,
                          kind="ExternalInput")
    zl_d = nc.dram_tensor("z_lower", [n_elems], F32, kind="ExternalOutput")
    zu_d = nc.dram_tensor("z_upper", [n_elems], F32, kind="ExternalOutput")

    # m-major: element e = p*M + (t*S + s)
    pr = pp_d[:].rearrange("(p t s) k -> t p s k", p=P, t=T, s=S)
    xlr = xl_d[:].rearrange("(p m) -> p m", p=P)
    xur = xu_d[:].rearrange("(p m) -> p m", p=P)
    zlr = zl_d[:].rearrange("(p m) -> p m", p=P)
    zur = zu_d[:].rearrange("(p m) -> p m", p=P)

    scan_relu = _register_dve_op("SCAN_RELU_SPLINE", _SCAN_RELU, subdim=True)
    recip = DO.RECIPROCAL_APPROX_FAST
    rc = DO.RECIP_APPROX_FAST_CONSTS

    with tile.TileContext(nc) as tc:
        with tc.tile_pool(name="cst", bufs=1) as cst, \
             tc.tile_pool(name="io", bufs=2) as io, \
             tc.tile_pool(name="wk", bufs=1) as wk, \
             tc.tile_pool(name="ac", bufs=2) as ac:
            # resident inputs / outputs ([128, M]: one contiguous run per
            # partition in DRAM -> minimal DMA descriptor count)
            xlf = cst.tile([P, M], F32, tag="xlf")
            nc.sync.dma_start(out=xlf[:], in_=xlr)
            xuf = cst.tile([P, M], F32, tag="xuf")
            nc.sync.dma_start(out=xuf[:], in_=xur)
            zall = cst.tile([P, 2, M], F32, tag="zall")
            # centered widths (w - 1/16) with a permanent 0 ahead of col 0:
            # the SCAN_RELU op reads the 1-shifted view => exclusive cumsum
            wt32e = cst.tile([P, S * K + 1], F32, tag="wt32e")
            nc.vector.memset(wt32e[:, 0:1], 0.0)
            wce_w = wt32e[:, 1:S * K + 1].rearrange("p (s k) -> p s k", k=K)
            wce_r = wt32e[:, 0:S * K].rearrange("p (s k) -> p s k", k=K)

            for t in range(T):
                sl = slice(t * S, (t + 1) * S)
                raw = io.tile([P, S, 34], F32, tag="raw")
                nc.sync.dma_start(out=raw[:], in_=pr[t])

                # =========== W path: fp32 =================================
                ew = ac.tile([P, S, K], F32, tag="ew")
                nc.scalar.activation(ew[:], raw[:, :, 0:K], ACT.Exp)
                Sw = wk.tile([P, S], F32, tag="Sw")
                nc.vector.reduce_sum(Sw[:], ew[:], axis=AX.X)
                # ~2-ULP reciprocal: the stream-scan relies on each segment
                # summing to 1 + O(ulp); the 51-ULP fast recip drifts ~2e-4
                # across 128 segments
                rSw = wk.tile([P, S], F32, tag="rSw")
                rSws = wk.tile([P, S], F32, tag="rSws")
                nc.vector.reciprocal_approx_accurate(rSw[:], Sw[:], rSws[:])
                nc.vector.tensor_scalar(rSw[:], rSw[:], SCALE, None, OP.mult)
                rSw_b = rSw[:].unsqueeze(2).broadcast_to([P, S, K])
                nc.vector.tensor_tensor(wce_w, ew[:], rSw_b, OP.mult)
                nc.scalar.activation(wce_w, wce_w, ACT.Copy,
                                     bias=MIN_BIN - 1.0 / K)
                # uncentered fp32 widths (for the reciprocal + fp16 copy)
                wt32 = wk.tile([P, S, K], F32, tag="wt32")
                nc.scalar.activation(wt32[:], wce_w, ACT.Copy, bias=1.0 / K)
                wt16 = wk.tile([P, S, K], F16, tag="wt16")
                nc.scalar.copy(wt16[:], wt32[:])
                rw32 = wk.tile([P, S, K], F32, tag="rw32")
                nc.vector.reciprocal_approx_fast(rw32[:], wt32[:])
                rw16 = wk.tile([P, S, K], F16, tag="rw16")
                nc.scalar.copy(rw16[:], rw32[:])

                # =========== H path: fp16 =================================
                eh = wk.tile([P, S, K], F16, tag="eh")
                nc.scalar.activation(eh[:], raw[:, :, K:2 * K], ACT.Exp)
                hs = wk.tile([P, S, K // 2], F16, tag="hs")
                nc.vector.tensor_tensor(hs[:], eh[:, :, 0:8], eh[:, :, 8:16],
                                        OP.add)
                nc.vector.tensor_tensor(hs[:, :, 0:4], hs[:, :, 0:4],
                                        hs[:, :, 4:8], OP.add)
                Sh = wk.tile([P, S], F32, tag="Sh")
                nc.vector.reduce_sum(Sh[:], hs[:, :, 0:4], axis=AX.X)
                rSh32 = wk.tile([P, S], F32, tag="rSh32")
                nc.vector.reciprocal_approx_fast(rSh32[:], Sh[:])
                rSh = wk.tile([P, S], F16, tag="rSh")
                nc.vector.tensor_scalar(rSh[:], rSh32[:], SCALE, None, OP.mult)
                ht = wk.tile([P, S, K], F16, tag="ht")
                rSh_b = rSh[:].unsqueeze(2).broadcast_to([P, S, K])
                nc.vector.tensor_tensor(ht[:], eh[:], rSh_b, OP.mult)
                nc.scalar.activation(ht[:], ht[:], ACT.Copy, bias=MIN_BIN)

                # =========== slopes + derivatives (fp16) ===================
                st_ = wk.tile([P, S, K], F16, tag="st")
                nc.vector.tensor_tensor(st_[:], ht[:], rw16[:], OP.mult)
                # aligned copies of the +1-shifted slices (ACT)
                stR = wk.tile([P, S, K], F16, tag="stR")
                nc.scalar.copy(stR[:, :, 0:K - 1], st_[:, :, 1:K])
                wtR = wk.tile([P, S, K], F16, tag="wtR")
                nc.scalar.copy(wtR[:, :, 0:K - 1], wt16[:, :, 1:K])
                sL = st_[:, :, 0:K - 1]
                wL = wt16[:, :, 0:K - 1]
                sR = stR[:, :, 0:K - 1]
                wR = wtR[:, :, 0:K - 1]
                m1 = wk.tile([P, S, K], F16, tag="m1")
                nc.vector.tensor_tensor(m1[:, :, 0:K - 1], sL, sR, OP.min)
                t1 = wk.tile([P, S, K], F16, tag="t1")
                nc.vector.tensor_tensor(t1[:, :, 0:K - 1], wR, sL, OP.mult)
                t2 = wk.tile([P, S, K], F16, tag="t2")
                nc.vector.tensor_tensor(t2[:, :, 0:K - 1], wL, sR, OP.mult)
                nc.vector.tensor_tensor(t1[:, :, 0:K - 1], t1[:, :, 0:K - 1],
                                        t2[:, :, 0:K - 1], OP.add)
                den16 = wk.tile([P, S, K], F16, tag="den16")
                nc.vector.tensor_tensor(den16[:, :, 0:K - 1], wL, wR, OP.add)
                # buffer reuse: wt32 is dead after wt16/rw16, t2 after the
                # t1+t2 fold
                den32 = wk.tile([P, S, K], F32, tag="wt32")
                nc.scalar.copy(den32[:, :, 0:K - 1], den16[:, :, 0:K - 1])
                rdn32 = wk.tile([P, S, K], F32, tag="rw32")
                nc.vector.reciprocal_approx_fast(rdn32[:, :, 0:K - 1],
                                                 den32[:, :, 0:K - 1])
                rdn16 = wk.tile([P, S, K], F16, tag="t2")
                nc.scalar.copy(rdn16[:, :, 0:K - 1], rdn32[:, :, 0:K - 1])
                nc.vector.tensor_tensor(t1[:, :, 0:K - 1], t1[:, :, 0:K - 1],
                                        rdn16[:, :, 0:K - 1], OP.mult)
                m1d = wk.tile([P, S, K], F16, tag="den16")
                nc.scalar.mul(m1d[:, :, 0:K - 1], m1[:, :, 0:K - 1], 2.0)
                # dlt padded to 18 so D0 slices stay 4B-aligned
                dlt = wk.tile([P, S, K + 2], F16, tag="dlt")
                nc.vector.tensor_tensor(dlt[:, :, 1:K], m1d[:, :, 0:K - 1],
                                        t1[:, :, 0:K - 1], OP.min)
                e01 = wk.tile([P, S, 2], F16, tag="e01")
                nc.scalar.activation(e01[:], raw[:, :, 2 * K:2 * K + 2],
                                     ACT.Tanh, scale=0.5)
                nc.vector.tensor_scalar(e01[:], e01[:], 1.5, 1.5,
                                        OP.mult, OP.add)
                nc.vector.tensor_tensor(dlt[:, :, 0:1], e01[:, :, 0:1],
                                        st_[:, :, 0:1], OP.mult)
                nc.vector.tensor_tensor(dlt[:, :, K:K + 1], e01[:, :, 1:2],
                                        st_[:, :, K - 1:K], OP.mult)

                # =========== Hermite coefficients ==========================
                D0 = dlt[:, :, 0:K]
                # aligned copy of D1 (ACT), then all coeff ops run 2x
                d1c = wk.tile([P, S, K], F16, tag="eh")
                nc.scalar.copy(d1c[:], dlt[:, :, 1:K + 1])
                # aN = 2st - D0 - D1 = (st-D0) + (st-D1); bc = aN + (st-D0)
                sm = wk.tile([P, S, K], F16, tag="sm")
                nc.vector.tensor_tensor(sm[:], st_[:], D0, OP.subtract)
                sm1 = wk.tile([P, S, K], F16, tag="stR")
                nc.vector.tensor_tensor(sm1[:], st_[:], d1c[:], OP.subtract)
                aN = wk.tile([P, S, K], F16, tag="aN")
                nc.vector.tensor_tensor(aN[:], sm[:], sm1[:], OP.add)
                bc = wk.tile([P, S, K], F16, tag="bc")
                nc.vector.tensor_tensor(bc[:], aN[:], sm[:], OP.add)

                # =========== evaluate both x in one [P,2,S,K] stream =======
                # fused custom op: ttr = relu(x - CW_{k-1}) straight from the
                # centered-width stream (scan + decentering + sub + relu).
                # fp32 out: the scan accumulator follows the output dtype.
                tt2 = wk.tile([P, 2, S, K], F16, tag="tt2")
                for j, xf in ((0, xlf), (1, xuf)):
                    x_b = xf[:, sl].unsqueeze(2).broadcast_to([P, S, K])
                    ttw = wk.tile([P, S, K], F32, tag=f"ttw{j}")
                    nc.vector._custom_dve(scan_relu, out=ttw[:],
                                          in0=wce_r, in1=x_b, s0=1.0 / K)
                    nc.scalar.copy(tt2[:, j], ttw[:])
                wt_b = wt16[:].unsqueeze(1).broadcast_to([P, 2, S, K])
                sg2 = wk.tile([P, 2, S, K], F16, tag="sg2")
                nc.vector.tensor_tensor(sg2[:], tt2[:], wt_b, OP.min)
                u2 = wk.tile([P, 2, S, K], F16, tag="tt2")
                rw_b = rw16[:].unsqueeze(1).broadcast_to([P, 2, S, K])
                nc.vector.tensor_tensor(u2[:], sg2[:], rw_b, OP.mult)
                aN_b = aN[:].unsqueeze(1).broadcast_to([P, 2, S, K])
                bc_b = bc[:].unsqueeze(1).broadcast_to([P, 2, S, K])
                D0_b = D0.unsqueeze(1).broadcast_to([P, 2, S, K])
                hv = wk.tile([P, 2, S, K], F16, tag="hv")
                nc.vector.tensor_tensor(hv[:], aN_b, u2[:], OP.mult)
                nc.vector.tensor_tensor(hv[:], bc_b, hv[:], OP.subtract)
                nc.vector.tensor_tensor(hv[:], hv[:], u2[:], OP.mult)
                nc.vector.tensor_tensor(hv[:], hv[:], D0_b, OP.add)
                nc.vector.tensor_tensor(hv[:], hv[:], sg2[:], OP.mult)
                # tree to 4, then one reduce into the resident z tile
                nc.vector.tensor_tensor(hv[:, :, :, 0:8], hv[:, :, :, 0:8],
                                        hv[:, :, :, 8:16], OP.add)
                nc.vector.tensor_tensor(hv[:, :, :, 0:4], hv[:, :, :, 0:4],
                                        hv[:, :, :, 4:8], OP.add)
                zt = zall[:, :, sl]
                nc.vector.reduce_sum(zt, hv[:, :, :, 0:4], axis=AX.X)
                nc.vector.tensor_scalar(zt, zt, 1.0, 0.0, OP.min, OP.max)

            nc.sync.dma_start(out=zlr, in_=zall[:, 0, :])
            nc.sync.dma_start(out=zur, in_=zall[:, 1, :])
    nc.finalize()
    return nc


_PROGRAM_CACHE = {}


def _get_program(n_elems, S=128):
    key = (n_elems, S)
    if key not in _PROGRAM_CACHE:
        _PROGRAM_CACHE[key] = build_program(n_elems, S)
    return _PROGRAM_CACHE[key]


def kernel(x_lower, x_upper, elementwise_params):
    x_lower = np.ascontiguousarray(x_lower, dtype=np.float32)
    x_upper = np.ascontiguousarray(x_upper, dtype=np.float32)
    elementwise_params = np.ascontiguousarray(elementwise_params,
                                              dtype=np.float32)
    Bb = x_lower.shape[0]
    per = Bb // N_CORES
    n_elems = per * C * H * W

    nc = _get_program(n_elems)
    in_maps = []
    for c in range(N_CORES):
        sl = slice(c * per, (c + 1) * per)
        in_maps.append({
            "x_lower": x_lower[sl].reshape(n_elems),
            "x_upper": x_upper[sl].reshape(n_elems),
            "elementwise_params": elementwise_params[sl].reshape(n_elems, 34),
        })
    res = run_bass_kernel_spmd(nc, in_maps, list(range(N_CORES)))
    zl = np.concatenate([r["z_lower"].reshape(per, C, H, W)
                         for r in res.results], axis=0)
    zu = np.concatenate([r["z_upper"].reshape(per, C, H, W)
                         for r in res.results], axis=0)
    return zl, zu


if __name__ == "__main__":
    rng = np.random.default_rng(0)
    xl = rng.random((B, C, H, W), dtype=np.float32)
    xu = rng.random((B, C, H, W), dtype=np.float32)
    pp = rng.standard_normal((B, C, H, W, 34), dtype=np.float32)
    zl, zu = kernel(x_lower=xl, x_upper=xu, elementwise_params=pp)
    print("ok", zl.shape, zu.shape, zl.min(), zl.max())
